# revision 53
# baseline (speedup 1.0000x reference)
"""ANFIS forward kernel for Trainium2, 8-core data-parallel.

Model (reference):
    mf        = exp(-(x-c)^2 / (2 s^2))            # (N,R,D)
    strengths = prod_d mf                          # (N,R)
    norm      = strengths / (sum_r strengths + eps)
    wi        = X @ coeffs[r,:-1] + coeffs[r,-1]   # (N,R,O)
    out       = softmax_o( sum_r norm * wi )       # (N,O)

Kernel algebra (per row n):
    l_r   = sum_d [ 2*c*a*x - a*x^2 ] - k_r,   a = 1/(2 s^2), k_r = sum_d c^2 a
    s_r   = exp(l_r)                            (strengths)
    G_io  = sum_r s_r * Chat[r,i,o]             (Chat = [coeffs[:, :16]; bias row; ones])
    U_o   = sum_i xhat_i * G_io                 (xhat = [x, 1])
    S     = sum_r s_r  (via ones feature)
    out   = softmax_o( U / (S + eps) )

Layout: supergroups of 2048 rows = 16 tiles of 128 rows; j in 0..3 indexes
row-tiles within a 512-row group, q in 0..3 indexes groups in a supergroup.
  - Host supplies X pre-transposed (partition=(j,d)) so no PE transpose;
    GpSimd squares it into partitions 64..127.
  - M1 (f32r, 1 cyc/col): one (128,512) matmul per supergroup ->
    logits l[(j,r),(q,p)] in PSUM.
  - exp on ACT with per-partition bias ln(SCALE)-k_r -> sst (fp16, SBUF).
    SCALE keeps strengths out of the fp16 subnormal range; it cancels in
    U/(S + SCALE*eps).
  - M2 (fp16): per group, lhsT=sst column block, rhs=block-diag Chat with
    feature order f = o*18 + i (i innermost), o in 0..10 (o=10 carries the
    strength-sum feature at i=16 where the xhat broadcast is 1).
  - ACT evicts G to fp16 SBUF so the DVE multiply runs in 2x mode.
  - DVE: P = G * x (bcast over o, unit-stride i) at 2x, then sum over i
    via a 2x-mode add tree (16->8->4->2->1) + bias/S column; eps is folded
    into the tail as a uniform logit shift (softmax-invariant).
  - Softmax batched over 16 tiles, software-pipelined one supergroup back.
All DRAM I/O is contiguous per partition (xa loads, const loads and output
stores ride the GpSimd SWDGE queue); host does the (cheap) unpermutes.
"""

import numpy as np

N, D, R, O = 131072, 16, 32, 10
EPS = 1e-8
NCORES = 8
MC = N // NCORES          # rows per core = 16384
TPG = 4                   # tiles (of 128 rows) per group
GROUP = 128 * TPG         # 512 rows per group
NG = MC // GROUP          # 32 groups per core
META = 4                  # groups per supergroup
NMETA = NG // META        # 8 supergroups
SGC = META * 128          # matmul columns per supergroup (512)

DI = D + 1                # 17: x dims + ones
DIP = DI + 1              # 18: padded i-stride (keeps fp16 runs 4B-aligned)
F = DI * O                # 170 product features
OS = O + 1                # 11 o-blocks: 10 outputs + 1 strength-sum block
FS = OS * DIP             # 198 feature cols; col 196 = (o=10,i=16) = S ones
FPAD = 256                # per-j feature stride in G (bank-friendly)
SCALE = 16384.0           # strength rescale to dodge fp16 subnormals


def _build_constants(centers, sigmas, coeffs):
    a = 1.0 / (2.0 * sigmas.astype(np.float64) ** 2)          # (R,D)
    c = centers.astype(np.float64)

    # WL: lhsT for M1. out partition (j,r) = j*32+r ; rhs partition (s,j,d).
    wl = np.zeros((128, 128), np.float64)
    for j in range(TPG):
        for r in range(R):
            pi = j * R + r
            for d in range(D):
                wl[0 * 64 + j * 16 + d, pi] = 2.0 * c[r, d] * a[r, d]   # x
                wl[1 * 64 + j * 16 + d, pi] = -a[r, d]                  # x^2
    # +ln(SCALE) rescales strengths away from the fp16 subnormal zone;
    # the factor cancels in U/(S + SCALE*eps).
    negk = -(c * c * a).sum(axis=1) + np.log(SCALE)            # (R,)
    negk4 = np.tile(negk, TPG).reshape(128, 1)

    # Chat (R, FS) with feature order f = o*18 + i (i innermost, 1 pad col).
    # The S-sum feature sits at (o=10, i=16): the xhat broadcast there is
    # the ones column, so P[.,10,16] = S survives the mult+reduce unchanged.
    chat = np.zeros((R, FS), np.float64)
    chat.reshape(R, OS, DIP)[:, :O, :DI] = (
        coeffs.astype(np.float64).transpose(0, 2, 1).reshape(R, O, DI))
    chat[:, O * DIP + D] = 1.0
    # c2d (128, 4*FPAD): [(j,r), j*FPAD + f] = chat[r,f]
    c2d = np.zeros((128, TPG * FPAD), np.float64)
    for j in range(TPG):
        c2d[j * R:(j + 1) * R, j * FPAD:j * FPAD + FS] = chat
    return (wl.astype(np.float32), negk4.astype(np.float32),
            c2d.astype(np.float32))


def _prepare_x(X):
    """Per-core host-side layouts (pure permutation / padding)."""
    X = np.asarray(X, np.float32)
    # row = ((m*META + q)*TPG + j)*128 + p
    xv = X.reshape(NCORES, NMETA, META, TPG, 128, D)
    # XT[(j,d), m*512 + q*128 + p]
    xt = np.ascontiguousarray(
        xv.transpose(0, 3, 5, 1, 2, 4).reshape(NCORES, 64, NMETA * SGC))
    # xa[p, (m, q, j, i)] fp16, ones at i=16, zero pad at i=17
    xa = np.zeros((NCORES, 128, NMETA, META, TPG, DIP), np.float16)
    xa[..., :D] = xv.transpose(0, 4, 1, 2, 3, 5)
    xa[..., D] = 1.0
    xa = np.ascontiguousarray(xa.reshape(NCORES, 128, NG * TPG * DIP))
    return xt, xa


def _unpermute_out(yts):
    """yts: list of (128, NG*TPG*O) per core -> (N, O)."""
    y = np.stack(yts, axis=0).reshape(NCORES, 128, NMETA, META, TPG, O)
    y = y.transpose(0, 2, 3, 4, 1, 5).reshape(N, O)
    return np.ascontiguousarray(y)


def _build_bass():
    import concourse.bacc as bacc
    import concourse.mybir as mybir
    from concourse.tile import TileContext

    f32 = mybir.dt.float32
    f32r = mybir.dt.float32r
    f16 = mybir.dt.float16
    AX = mybir.AxisListType
    ALU = mybir.AluOpType
    ACTF = mybir.ActivationFunctionType

    nc = bacc.Bacc("TRN2", target_bir_lowering=False, debug=False)
    xt_d = nc.declare_dram_parameter("xt", [64, NMETA * SGC], f32r,
                                     isOutput=False)
    xa_d = nc.declare_dram_parameter("xa", [128, NG * TPG * DIP], f16,
                                     isOutput=False)
    wl_d = nc.declare_dram_parameter("wl", [128, 128], f32r, isOutput=False)
    negk4_d = nc.declare_dram_parameter("negk4", [128, 1], f32,
                                        isOutput=False)
    c2d_d = nc.declare_dram_parameter("c2d", [128, TPG * FPAD], f16,
                                      isOutput=False)
    yt_d = nc.declare_dram_parameter("yt", [128, NG * TPG * O], f32,
                                     isOutput=True)

    with TileContext(nc) as tc:
        with (
            tc.tile_pool(name="const", bufs=1) as cpool,
            tc.tile_pool(name="xin", bufs=3) as xpool,
            tc.tile_pool(name="work", bufs=4) as wpool,
            tc.tile_pool(name="sm", bufs=3) as spool,
            tc.tile_pool(name="wide", bufs=2) as wpool2,
            tc.tile_pool(name="ps_l", bufs=2, space="PSUM") as ps_l,
            tc.tile_pool(name="ps_g", bufs=3, space="PSUM") as ps_g,
        ):
            # Trigger the ACT exp table load immediately so its ~2.7us
            # cost overlaps the preamble instead of the first supergroup.
            warm = cpool.tile([128, 2], f32)
            nc.vector.memset(warm[:], 0.0)
            nc.scalar.activation(warm[:], warm[:], ACTF.Exp)
            wl = cpool.tile([128, 128], f32r)
            nc.gpsimd.dma_start(out=wl[:], in_=wl_d[:, :])
            negk4 = cpool.tile([128, 1], f32)
            nc.gpsimd.dma_start(out=negk4[:], in_=negk4_d[:, :])
            c2d = cpool.tile([128, TPG * FPAD], f16)
            nc.gpsimd.dma_start(out=c2d[:], in_=c2d_d[:, :])

            NB2 = 2 * META * TPG

            def _softmax_store(mp, u16d):
                # softmax + store for a PAIR of supergroups (32 tiles)
                e16 = spool.tile([128, NB2 * OS], f32, tag="e16")
                se16 = spool.tile([128, NB2], f32, tag="se16")
                o16 = spool.tile([128, NB2 * O], f32, tag="o16")
                u16v = u16d[:].rearrange("p (g o) -> p g o", o=OS)
                sv = u16v[:, :, O:OS]
                nc.vector.tensor_scalar_add(sv, sv, EPS * SCALE)
                nc.vector.reciprocal_approx_fast(sv, sv)
                s16b = sv.broadcast_to([128, NB2, O])
                nc.vector.tensor_tensor(u16v[:, :, 0:O], u16v[:, :, 0:O],
                                        s16b, ALU.mult)
                nc.scalar.activation(e16[:], u16d[:], ACTF.Exp)
                e16v = e16[:].rearrange("p (g o) -> p g o", o=OS)
                nc.vector.tensor_reduce(
                    se16[:], e16v[:, :, 0:O], axis=AX.X, op=ALU.add)
                nc.vector.reciprocal_approx_fast(se16[:], se16[:])
                se16b = se16[:].unsqueeze(2).broadcast_to([128, NB2, O])
                nc.vector.tensor_tensor(
                    o16[:].rearrange("p (g o) -> p g o", o=O),
                    e16v[:, :, 0:O],
                    se16b, ALU.mult)
                nc.gpsimd.dma_start(
                    out=yt_d[:, mp * NB2 * O:(mp + 1) * NB2 * O],
                    in_=o16[:])

            pend = []
            XAW = META * TPG * DIP
            for m in range(NMETA):
                # ---- load transposed x, square into lower half ----------
                # sgs 0/1 load singly (shorter pipeline fill, first square
                # on ACT); later sgs load in pairs with GpSimd squares.
                if m < 2:
                    xs1 = xpool.tile([128, SGC], f32r, tag="xs1")
                    if m == 0:
                        for hh in range(2):
                            cs = slice(hh * 256, (hh + 1) * 256)
                            nc.sync.dma_start(
                                out=xs1[0:64, cs],
                                in_=xt_d[:, m * SGC + hh * 256:
                                         m * SGC + (hh + 1) * 256])
                    else:
                        nc.sync.dma_start(out=xs1[0:64, :],
                                          in_=xt_d[:, m * SGC:(m + 1) * SGC])
                    xa1 = xpool.tile([128, XAW], f16, tag="xa1")
                    nc.gpsimd.dma_start(out=xa1[:],
                                        in_=xa_d[:, m * XAW:(m + 1) * XAW])
                    if m == 0:
                        # DVE is idle during the fill; square there so it
                        # overlaps the ACT table load. Halves are squared
                        # as their DMA chunks land.
                        for hh in range(2):
                            cs = slice(hh * 256, (hh + 1) * 256)
                            nc.vector.tensor_tensor(
                                xs1[64:128, cs], xs1[0:64, cs],
                                xs1[0:64, cs], ALU.mult)
                    else:
                        nc.gpsimd.tensor_tensor(
                            xs1[64:128, :], xs1[0:64, :], xs1[0:64, :],
                            ALU.mult)
                    xs = xs1[:, :]
                    xa = xa1[:, :]
                else:
                    if m % 2 == 0:
                        xs2 = xpool.tile([128, 2 * SGC], f32r, tag="xs")
                        nc.sync.dma_start(out=xs2[0:64, :],
                                          in_=xt_d[:, m * SGC:(m + 2) * SGC])
                        xa2 = xpool.tile([128, 2 * XAW], f16, tag="xa")
                        nc.gpsimd.dma_start(
                            out=xa2[:],
                            in_=xa_d[:, m * XAW:(m + 2) * XAW])
                        nc.gpsimd.tensor_tensor(xs2[64:128, :], xs2[0:64, :],
                                                xs2[0:64, :], ALU.mult)
                    h2 = m % 2
                    xs = xs2[:, h2 * SGC:(h2 + 1) * SGC]
                    xa = xa2[:, h2 * XAW:(h2 + 1) * XAW]
                # ---- M1: logits for the whole supergroup ----------------
                lt = ps_l.tile([128, SGC], f32, tag="l")
                l = lt[:, 0:SGC]
                if m == 0:
                    for hh in range(2):
                        cs = slice(hh * 256, (hh + 1) * 256)
                        nc.tensor.matmul(l[:, cs], lhsT=wl[:],
                                         rhs=xs[:, cs],
                                         start=True, stop=True)
                else:
                    nc.tensor.matmul(l, lhsT=wl[:], rhs=xs,
                                     start=True, stop=True)
                # ---- strengths = exp(l - k) -----------------------------
                sst = wpool.tile([128, SGC], f16, tag="sst")
                nc.scalar.activation(sst[:], l, ACTF.Exp,
                                     bias=negk4[:, 0:1], scale=1.0)

                if m % 2 == 0:
                    u16d = spool.tile([128, NB2 * OS], f32, tag="u16")
                u16 = u16d[:, (m % 2) * META * TPG * OS:
                           (m % 2 + 1) * META * TPG * OS]

                NB = META * TPG
                if m < 2:
                    gh = wpool.tile([128, NB * FS], f16, tag="gh1")
                else:
                    if m % 2 == 0:
                        gh2 = wpool2.tile([128, 2 * NB * FS], f16, tag="gh")
                    gh = gh2[:, (m % 2) * NB * FS:(m % 2 + 1) * NB * FS]
                for q in range(META):
                    # -- M2: G = strengths-weighted consequent sums.
                    # j-blocks at stride FPAD=256; each 454-col matmul
                    # sits inside one PSUM bank. --------------------------
                    g4 = ps_g.tile([128, TPG * FPAD], f32, tag="g4")
                    w2 = FPAD + FS
                    lq = sst[:, q * 128:(q + 1) * 128]
                    nc.tensor.matmul(g4[:, 0:w2], lhsT=lq,
                                     rhs=c2d[:, 0:w2],
                                     start=True, stop=True)
                    nc.tensor.matmul(g4[:, 512:512 + w2],
                                     lhsT=lq, rhs=c2d[:, 512:512 + w2],
                                     start=True, stop=True)
                    # -- evict G to fp16 SBUF (enables 2x DVE mult);
                    # only the used (o,i) region, pad columns skipped -----
                    nc.scalar.activation(
                        gh[:, q * TPG * FS:(q + 1) * TPG * FS].rearrange(
                            "p (j o i) -> p j o i",
                            j=TPG, o=OS)[:, :, :, 0:DI],
                        g4[:].rearrange("p (j f) -> p j f",
                                        j=TPG)[:, :, 0:FS].rearrange(
                            "p j (o i) -> p j o i", i=DIP)[:, :, :, 0:DI],
                        ACTF.Copy)
                # -- P = G * x (bcast over o, unit-stride i) --------------
                if m >= 2 and m % 2 == 0:
                    continue        # DVE work emitted pair-wide at odd m
                if m >= 2:
                    gh = gh2[:, :]
                    xa = xa2[:, :]
                    u16 = u16d[:, :]
                    NB = 2 * META * TPG
                ghv = gh.rearrange("p (g o i) -> p g o i", g=NB, o=OS)
                wtag = "w" if NB > META * TPG else "n"
                pp = wpool2 if NB > META * TPG else wpool
                p4 = pp.tile([128, NB * O * D], f16, tag="p4" + wtag)
                p4v = p4[:].rearrange("p (g o i) -> p g o i", g=NB, o=O)
                xhv = xa.rearrange(
                    "p (g i) -> p g i", i=DIP)[:, :, 0:D].unsqueeze(
                    2).broadcast_to([128, NB, O, D])
                nsp = 4 if m == 0 else 2
                HB = NB // nsp
                for h in range(nsp):
                    sl = slice(h * HB, (h + 1) * HB)
                    nc.vector.tensor_tensor(
                        p4v[:, sl], ghv[:, sl, 0:O, 0:D], xhv[:, sl],
                        ALU.mult)
                # -- U = sum_i P via 2x-mode add tree; the (i=16) column
                # (bias terms, and S in o-block 10) joins at the end -------
                t1 = pp.tile([128, NB * O * 8], f16, tag="t1" + wtag)
                t1v = t1[:].rearrange("p (g o i) -> p g o i", g=NB, o=O)
                nc.vector.tensor_tensor(t1v, p4v[:, :, :, 0:8],
                                        p4v[:, :, :, 8:16], ALU.add)
                t2 = pp.tile([128, NB * O * 4], f16, tag="t2" + wtag)
                t2v = t2[:].rearrange("p (g o i) -> p g o i", g=NB, o=O)
                nc.vector.tensor_tensor(t2v, t1v[:, :, :, 0:4],
                                        t1v[:, :, :, 4:8], ALU.add)
                t3 = pp.tile([128, NB * O * 2], f16, tag="t3" + wtag)
                t3v = t3[:].rearrange("p (g o i) -> p g o i", g=NB, o=O)
                nc.vector.tensor_tensor(t3v, t2v[:, :, :, 0:2],
                                        t2v[:, :, :, 2:4], ALU.add)
                u16v3 = u16.rearrange("p (g o) -> p g o", o=OS)
                nc.vector.tensor_tensor(
                    u16v3[:, :, 0:O], t3v[:, :, :, 0:1].squeeze(3),
                    ghv[:, :, 0:O, D:D + 1].squeeze(3), ALU.add)
                nc.vector.tensor_tensor(
                    u16v3[:, :, 0:O], u16v3[:, :, 0:O],
                    t3v[:, :, :, 1:2].squeeze(3), ALU.add)
                nc.vector.tensor_copy(
                    u16v3[:, :, O:OS],
                    ghv[:, :, O:OS, D:D + 1].squeeze(3))

                # -- softmax of the PREVIOUS supergroup (software-
                # pipelined so it never head-of-line-blocks the ACT FIFO) --
                if m % 2 == 1:
                    pend.append((m // 2, u16d))
                    if len(pend) > 1:
                        _softmax_store(*pend.pop(0))
            _softmax_store(*pend.pop(0))
    nc.compile()
    return nc


_NC_CACHE = None


def _prepare_in_maps(X, centers, sigmas, coeffs):
    wl, negk4, c2d = _build_constants(
        np.asarray(centers, np.float32),
        np.asarray(sigmas, np.float32),
        np.asarray(coeffs, np.float32))
    xt, xa = _prepare_x(X)
    in_maps = []
    for c in range(NCORES):
        in_maps.append({
            "xt": xt[c], "xa": xa[c],
            "wl": wl, "negk4": negk4, "c2d": c2d.astype(np.float16),
        })
    return in_maps


def kernel(X, centers, sigmas, coeffs):
    global _NC_CACHE
    from concourse import bass_utils

    if _NC_CACHE is None:
        _NC_CACHE = _build_bass()
    nc = _NC_CACHE

    in_maps = _prepare_in_maps(X, centers, sigmas, coeffs)
    res = bass_utils.run_bass_kernel_spmd(nc, in_maps, list(range(NCORES)))
    return _unpermute_out([r["yt"] for r in res.results])


# revision 54
# speedup vs baseline: 1.2132x; 1.2132x over previous
"""ANFIS forward kernel for Trainium2, 8-core data-parallel.

Model (reference):
    mf        = exp(-(x-c)^2 / (2 s^2))            # (N,R,D)
    strengths = prod_d mf                          # (N,R)
    norm      = strengths / (sum_r strengths + eps)
    wi        = X @ coeffs[r,:-1] + coeffs[r,-1]   # (N,R,O)
    out       = softmax_o( sum_r norm * wi )       # (N,O)

Kernel algebra (per row n):
    l_r   = sum_d [ 2*c*a*x - a*x^2 ] - k_r,   a = 1/(2 s^2), k_r = sum_d c^2 a
    s_r   = exp(l_r)                            (strengths)
    G_io  = sum_r s_r * Chat[r,i,o]             (Chat = [coeffs[:, :16]; bias row; ones])
    U_o   = sum_i xhat_i * G_io                 (xhat = [x, 1])
    S     = sum_r s_r  (via ones feature)
    out   = softmax_o( U / (S + eps) )

Layout: supergroups of 2048 rows = 16 tiles of 128 rows; j in 0..3 indexes
row-tiles within a 512-row group, q in 0..3 indexes groups in a supergroup.
  - Host supplies X pre-transposed (partition=(j,d)) so no PE transpose;
    GpSimd squares it into partitions 64..127.
  - M1 (f32r, 1 cyc/col): one (128,512) matmul per supergroup ->
    logits l[(j,r),(q,p)] in PSUM.
  - exp on ACT with per-partition bias ln(SCALE)-k_r -> sst (fp16, SBUF).
    SCALE keeps strengths out of the fp16 subnormal range; it cancels in
    U/(S + SCALE*eps).
  - M2 (fp16): per group, lhsT=sst column block, rhs=block-diag Chat with
    feature order f = o*18 + i (i innermost), o in 0..10 (o=10 carries the
    strength-sum feature at i=16 where the xhat broadcast is 1).
  - ACT evicts G to fp16 SBUF so the DVE multiply runs in 2x mode.
  - DVE: P = G * x (bcast over o, unit-stride i) at 2x, then sum over i
    via a 2x-mode add tree (16->8->4->2->1) + bias/S column; eps is folded
    into the tail as a uniform logit shift (softmax-invariant).
  - Softmax batched over 16 tiles, software-pipelined one supergroup back.
All DRAM I/O is contiguous per partition (xa loads, const loads and output
stores ride the GpSimd SWDGE queue); host does the (cheap) unpermutes.
"""

import numpy as np

N, D, R, O = 131072, 16, 32, 10
EPS = 1e-8
NCORES = 8
MC = N // NCORES          # rows per core = 16384
TPG = 4                   # tiles (of 128 rows) per group
GROUP = 128 * TPG         # 512 rows per group
NG = MC // GROUP          # 32 groups per core
META = 4                  # groups per supergroup
NMETA = NG // META        # 8 supergroups
SGC = META * 128          # matmul columns per supergroup (512)

DI = D + 1                # 17: x dims + ones
DIP = DI + 1              # 18: padded i-stride (keeps fp16 runs 4B-aligned)
F = DI * O                # 170 product features
OS = O + 1                # 11 o-blocks: 10 outputs + 1 strength-sum block
FS = OS * DIP             # 198 feature cols; col 196 = (o=10,i=16) = S ones
FPAD = 256                # per-j feature stride in G (bank-friendly)
SCALE = 16384.0           # strength rescale to dodge fp16 subnormals


def _build_constants(centers, sigmas, coeffs):
    a = 1.0 / (2.0 * sigmas.astype(np.float64) ** 2)          # (R,D)
    c = centers.astype(np.float64)

    # WL: lhsT for M1. out partition (j,r) = j*32+r ; rhs partition (s,j,d).
    wl = np.zeros((128, 128), np.float64)
    for j in range(TPG):
        for r in range(R):
            pi = j * R + r
            for d in range(D):
                wl[0 * 64 + j * 16 + d, pi] = 2.0 * c[r, d] * a[r, d]   # x
                wl[1 * 64 + j * 16 + d, pi] = -a[r, d]                  # x^2
    # +ln(SCALE) rescales strengths away from the fp16 subnormal zone;
    # the factor cancels in U/(S + SCALE*eps).
    negk = -(c * c * a).sum(axis=1) + np.log(SCALE)            # (R,)
    negk4 = np.tile(negk, TPG).reshape(128, 1)

    # Chat (R, FS) with feature order f = o*18 + i (i innermost, 1 pad col).
    # The S-sum feature sits at (o=10, i=16): the xhat broadcast there is
    # the ones column, so P[.,10,16] = S survives the mult+reduce unchanged.
    chat = np.zeros((R, FS), np.float64)
    chat.reshape(R, OS, DIP)[:, :O, :DI] = (
        coeffs.astype(np.float64).transpose(0, 2, 1).reshape(R, O, DI))
    chat[:, O * DIP + D] = 1.0
    # c2d (128, 4*FPAD): [(j,r), j*FPAD + f] = chat[r,f]
    c2d = np.zeros((128, TPG * FPAD), np.float64)
    for j in range(TPG):
        c2d[j * R:(j + 1) * R, j * FPAD:j * FPAD + FS] = chat
    return (wl.astype(np.float32), negk4.astype(np.float32),
            c2d.astype(np.float32))


def _prepare_x(X):
    """Per-core host-side layouts (pure permutation / padding)."""
    X = np.asarray(X, np.float32)
    # row = ((m*META + q)*TPG + j)*128 + p
    xv = X.reshape(NCORES, NMETA, META, TPG, 128, D)
    # XT[(j,d), m*512 + q*128 + p]
    xt = np.ascontiguousarray(
        xv.transpose(0, 3, 5, 1, 2, 4).reshape(NCORES, 64, NMETA * SGC))
    # xa[p, (m, q, j, i)] fp16, ones at i=16, zero pad at i=17
    xa = np.zeros((NCORES, 128, NMETA, META, TPG, DIP), np.float16)
    xa[..., :D] = xv.transpose(0, 4, 1, 2, 3, 5)
    xa[..., D] = 1.0
    xa = np.ascontiguousarray(xa.reshape(NCORES, 128, NG * TPG * DIP))
    return xt, xa


def _unpermute_out(yts):
    """yts: list of (128, NG*TPG*O) per core -> (N, O)."""
    y = np.stack(yts, axis=0).reshape(NCORES, 128, NMETA, META, TPG, O)
    y = y.transpose(0, 2, 3, 4, 1, 5).reshape(N, O)
    return np.ascontiguousarray(y)


def _build_bass():
    import concourse.bacc as bacc
    import concourse.mybir as mybir
    from concourse.tile import TileContext

    f32 = mybir.dt.float32
    f32r = mybir.dt.float32r
    f16 = mybir.dt.float16
    AX = mybir.AxisListType
    ALU = mybir.AluOpType
    ACTF = mybir.ActivationFunctionType

    nc = bacc.Bacc("TRN2", target_bir_lowering=False, debug=False)
    xt_d = nc.declare_dram_parameter("xt", [64, NMETA * SGC], f32r,
                                     isOutput=False)
    xa_d = nc.declare_dram_parameter("xa", [128, NG * TPG * DIP], f16,
                                     isOutput=False)
    wl_d = nc.declare_dram_parameter("wl", [128, 128], f32r, isOutput=False)
    negk4_d = nc.declare_dram_parameter("negk4", [128, 1], f32,
                                        isOutput=False)
    c2d_d = nc.declare_dram_parameter("c2d", [128, TPG * FPAD], f16,
                                      isOutput=False)
    yt_d = nc.declare_dram_parameter("yt", [128, NG * TPG * O], f32,
                                     isOutput=True)

    with TileContext(nc) as tc:
        with (
            tc.tile_pool(name="const", bufs=1) as cpool,
            tc.tile_pool(name="xin", bufs=3) as xpool,
            tc.tile_pool(name="work", bufs=4) as wpool,
            tc.tile_pool(name="sm", bufs=3) as spool,
            tc.tile_pool(name="ps_l", bufs=2, space="PSUM") as ps_l,
            tc.tile_pool(name="ps_g", bufs=3, space="PSUM") as ps_g,
        ):
            # Trigger the ACT exp table load immediately so its ~2.7us
            # cost overlaps the preamble instead of the first supergroup.
            warm = cpool.tile([128, 2], f32)
            nc.vector.memset(warm[:], 0.0)
            nc.scalar.activation(warm[:], warm[:], ACTF.Exp)
            wl = cpool.tile([128, 128], f32r)
            nc.gpsimd.dma_start(out=wl[:], in_=wl_d[:, :])
            negk4 = cpool.tile([128, 1], f32)
            nc.gpsimd.dma_start(out=negk4[:], in_=negk4_d[:, :])
            c2d = cpool.tile([128, TPG * FPAD], f16)
            nc.gpsimd.dma_start(out=c2d[:], in_=c2d_d[:, :])

            NB2 = 2 * META * TPG

            def _softmax_store(mp, u16d):
                # softmax + store for a PAIR of supergroups (32 tiles)
                e16 = spool.tile([128, NB2 * OS], f32, tag="e16")
                se16 = spool.tile([128, NB2], f32, tag="se16")
                o16 = spool.tile([128, NB2 * O], f32, tag="o16")
                u16v = u16d[:].rearrange("p (g o) -> p g o", o=OS)
                sv = u16v[:, :, O:OS]
                nc.vector.tensor_scalar_add(sv, sv, EPS * SCALE)
                nc.vector.reciprocal_approx_fast(sv, sv)
                s16b = sv.broadcast_to([128, NB2, O])
                nc.vector.tensor_tensor(u16v[:, :, 0:O], u16v[:, :, 0:O],
                                        s16b, ALU.mult)
                nc.scalar.activation(e16[:], u16d[:], ACTF.Exp)
                e16v = e16[:].rearrange("p (g o) -> p g o", o=OS)
                nc.vector.tensor_reduce(
                    se16[:], e16v[:, :, 0:O], axis=AX.X, op=ALU.add)
                nc.vector.reciprocal_approx_fast(se16[:], se16[:])
                se16b = se16[:].unsqueeze(2).broadcast_to([128, NB2, O])
                nc.vector.tensor_tensor(
                    o16[:].rearrange("p (g o) -> p g o", o=O),
                    e16v[:, :, 0:O],
                    se16b, ALU.mult)
                nc.gpsimd.dma_start(
                    out=yt_d[:, mp * NB2 * O:(mp + 1) * NB2 * O],
                    in_=o16[:])

            pend = []
            XAW = META * TPG * DIP
            for m in range(NMETA):
                # ---- load transposed x, square into lower half ----------
                # sgs 0/1 load singly (shorter pipeline fill, first square
                # on ACT); later sgs load in pairs with GpSimd squares.
                if m < 2:
                    xs1 = xpool.tile([128, SGC], f32r, tag="xs1")
                    if m == 0:
                        for hh in range(2):
                            cs = slice(hh * 256, (hh + 1) * 256)
                            nc.sync.dma_start(
                                out=xs1[0:64, cs],
                                in_=xt_d[:, m * SGC + hh * 256:
                                         m * SGC + (hh + 1) * 256])
                    else:
                        nc.sync.dma_start(out=xs1[0:64, :],
                                          in_=xt_d[:, m * SGC:(m + 1) * SGC])
                    xa1 = xpool.tile([128, XAW], f16, tag="xa1")
                    nc.gpsimd.dma_start(out=xa1[:],
                                        in_=xa_d[:, m * XAW:(m + 1) * XAW])
                    if m == 0:
                        # DVE is idle during the fill; square there so it
                        # overlaps the ACT table load. Halves are squared
                        # as their DMA chunks land.
                        for hh in range(2):
                            cs = slice(hh * 256, (hh + 1) * 256)
                            nc.vector.tensor_tensor(
                                xs1[64:128, cs], xs1[0:64, cs],
                                xs1[0:64, cs], ALU.mult)
                    else:
                        nc.gpsimd.tensor_tensor(
                            xs1[64:128, :], xs1[0:64, :], xs1[0:64, :],
                            ALU.mult)
                    xs = xs1[:, :]
                    xa = xa1[:, :]
                else:
                    if m % 2 == 0:
                        xs2 = xpool.tile([128, 2 * SGC], f32r, tag="xs")
                        nc.sync.dma_start(out=xs2[0:64, :],
                                          in_=xt_d[:, m * SGC:(m + 2) * SGC])
                        xa2 = xpool.tile([128, 2 * XAW], f16, tag="xa")
                        nc.gpsimd.dma_start(
                            out=xa2[:],
                            in_=xa_d[:, m * XAW:(m + 2) * XAW])
                        nc.gpsimd.tensor_tensor(xs2[64:128, :], xs2[0:64, :],
                                                xs2[0:64, :], ALU.mult)
                    h2 = m % 2
                    xs = xs2[:, h2 * SGC:(h2 + 1) * SGC]
                    xa = xa2[:, h2 * XAW:(h2 + 1) * XAW]
                # ---- M1: logits for the whole supergroup ----------------
                lt = ps_l.tile([128, SGC], f32, tag="l")
                l = lt[:, 0:SGC]
                if m == 0:
                    for hh in range(2):
                        cs = slice(hh * 256, (hh + 1) * 256)
                        nc.tensor.matmul(l[:, cs], lhsT=wl[:],
                                         rhs=xs[:, cs],
                                         start=True, stop=True)
                else:
                    nc.tensor.matmul(l, lhsT=wl[:], rhs=xs,
                                     start=True, stop=True)
                # ---- strengths = exp(l - k) -----------------------------
                sst = wpool.tile([128, SGC], f16, tag="sst")
                nc.scalar.activation(sst[:], l, ACTF.Exp,
                                     bias=negk4[:, 0:1], scale=1.0)

                if m % 2 == 0:
                    u16d = spool.tile([128, NB2 * OS], f32, tag="u16")
                u16 = u16d[:, (m % 2) * META * TPG * OS:
                           (m % 2 + 1) * META * TPG * OS]

                NB = META * TPG
                gh = wpool.tile([128, NB * FS], f16, tag="gh")
                for q in range(META):
                    # -- M2: G = strengths-weighted consequent sums.
                    # j-blocks at stride FPAD=256; each 454-col matmul
                    # sits inside one PSUM bank. --------------------------
                    g4 = ps_g.tile([128, TPG * FPAD], f32, tag="g4")
                    w2 = FPAD + FS
                    lq = sst[:, q * 128:(q + 1) * 128]
                    nc.tensor.matmul(g4[:, 0:w2], lhsT=lq,
                                     rhs=c2d[:, 0:w2],
                                     start=True, stop=True)
                    nc.tensor.matmul(g4[:, 512:512 + w2],
                                     lhsT=lq, rhs=c2d[:, 512:512 + w2],
                                     start=True, stop=True)
                    # -- evict G to fp16 SBUF (enables 2x DVE mult);
                    # only the used (o,i) region, pad columns skipped -----
                    nc.scalar.activation(
                        gh[:, q * TPG * FS:(q + 1) * TPG * FS
                           ].rearrange("p (j o i) -> p j o i",
                                       j=TPG, o=OS)[:, :, :, 0:DI],
                        g4[:].rearrange("p (j f) -> p j f",
                                        j=TPG)[:, :, 0:FS].rearrange(
                            "p j (o i) -> p j o i", i=DIP)[:, :, :, 0:DI],
                        ACTF.Copy)
                # -- P = G * x (bcast over o, unit-stride i), per pair ----
                ghv = gh[:].rearrange("p (g o i) -> p g o i", g=NB, o=OS)
                p4 = wpool.tile([128, NB * O * D], f16, tag="p4")
                p4v = p4[:].rearrange("p (g o i) -> p g o i", g=NB, o=O)
                xhv = xa.rearrange(
                    "p (g i) -> p g i", i=DIP)[:, :, 0:D].unsqueeze(
                    2).broadcast_to([128, NB, O, D])
                nsp = 4 if m == 0 else 2
                HB = NB // nsp
                for h in range(nsp):
                    sl = slice(h * HB, (h + 1) * HB)
                    nc.vector.tensor_tensor(
                        p4v[:, sl], ghv[:, sl, 0:O, 0:D], xhv[:, sl],
                        ALU.mult)
                # -- U = sum_i P via 2x-mode add tree; the (i=16) column
                # (bias terms, and S in o-block 10) joins at the end -------
                t1 = wpool.tile([128, NB * O * 8], f16, tag="t1")
                t1v = t1[:].rearrange("p (g o i) -> p g o i", g=NB, o=O)
                nc.vector.tensor_tensor(t1v, p4v[:, :, :, 0:8],
                                        p4v[:, :, :, 8:16], ALU.add)
                t2 = wpool.tile([128, NB * O * 4], f16, tag="t2")
                t2v = t2[:].rearrange("p (g o i) -> p g o i", g=NB, o=O)
                nc.vector.tensor_tensor(t2v, t1v[:, :, :, 0:4],
                                        t1v[:, :, :, 4:8], ALU.add)
                t3 = wpool.tile([128, NB * O * 2], f16, tag="t3")
                t3v = t3[:].rearrange("p (g o i) -> p g o i", g=NB, o=O)
                nc.vector.tensor_tensor(t3v, t2v[:, :, :, 0:2],
                                        t2v[:, :, :, 2:4], ALU.add)
                u16v3 = u16.rearrange("p (g o) -> p g o", o=OS)
                nc.vector.tensor_tensor(
                    u16v3[:, :, 0:O], t3v[:, :, :, 0:1].squeeze(3),
                    ghv[:, :, 0:O, D:D + 1].squeeze(3), ALU.add)
                nc.vector.tensor_tensor(
                    u16v3[:, :, 0:O], u16v3[:, :, 0:O],
                    t3v[:, :, :, 1:2].squeeze(3), ALU.add)
                nc.vector.tensor_copy(
                    u16v3[:, :, O:OS],
                    ghv[:, :, O:OS, D:D + 1].squeeze(3))

                # -- softmax of the PREVIOUS supergroup (software-
                # pipelined so it never head-of-line-blocks the ACT FIFO) --
                if m % 2 == 1:
                    pend.append((m // 2, u16d))
                    if len(pend) > 1:
                        _softmax_store(*pend.pop(0))
            _softmax_store(*pend.pop(0))
    nc.compile()
    return nc


_NC_CACHE = None


def _prepare_in_maps(X, centers, sigmas, coeffs):
    wl, negk4, c2d = _build_constants(
        np.asarray(centers, np.float32),
        np.asarray(sigmas, np.float32),
        np.asarray(coeffs, np.float32))
    xt, xa = _prepare_x(X)
    in_maps = []
    for c in range(NCORES):
        in_maps.append({
            "xt": xt[c], "xa": xa[c],
            "wl": wl, "negk4": negk4, "c2d": c2d.astype(np.float16),
        })
    return in_maps


def kernel(X, centers, sigmas, coeffs):
    global _NC_CACHE
    from concourse import bass_utils

    if _NC_CACHE is None:
        _NC_CACHE = _build_bass()
    nc = _NC_CACHE

    in_maps = _prepare_in_maps(X, centers, sigmas, coeffs)
    res = bass_utils.run_bass_kernel_spmd(nc, in_maps, list(range(NCORES)))
    return _unpermute_out([r["yt"] for r in res.results])
